# revision 53
# baseline (speedup 1.0000x reference)
"""Trainium2 Bass kernel for the diagonal complex linear recurrence (SSM scan).

Problem: out[t, d] = z_d * out[t-1, d] + x[t, d],  z_d = exp(-exp(size_d) + i*theta_d)
         x: [T=8192, D=2048] f32, out: [T, D] complex64.

Strategy (v2.2, all-fp16 datapath):
  - Shard channels D across 8 cores (256 each), pure model parallelism.
  - Per core, layout [channels(partitions), time(free)].  The complex scan is
    decomposed per time-chunk of length L via a local phase twist:
        v[jL+l] = e^{i*theta*l} * W_j[l]
        W_j[l]  = r * W_j[l-1] + e^{-i*theta*l} * x[jL+l],   r = |z| (real)
    which splits into two REAL first-order scans (re/im) on the DVE
    tensor_tensor_scan (fp32 internal state; fp16 stored W).  The scan
    multiplier is a stride-0 broadcast AP of an exact f32 r column - no
    full [P, L] multiplier table needed.
  - Carry across chunks: K_j = e^{i*theta*L} * W_{j-1}[L-1], on ScalarE.
  - Twist + half the untwist products on DVE (fp16 2x packed mode); the
    other two untwist products on GPSIMD; the complex-mul adds ride the PE
    as identity-matmul PSUM accumulation (+I and -I stationaries so only
    {cos, -sin} tables are needed); ScalarE copies PSUM->SBUF fp16.
  - fp16 I/O halves HBM traffic: x in fp16, out re/im in fp16.
"""

import os
import sys

import numpy as np

for _p in ("/opt/trn_rl_repo", "/root/.axon_site/_ro/trn_rl_repo"):
    if os.path.isdir(_p) and _p not in sys.path:
        sys.path.append(_p)

import concourse.bacc as bacc
import concourse.mybir as mybir
from concourse import bass_utils
from concourse.tile import TileContext

T = 8192
D = 2048
NCORES = 8
DS = D // NCORES          # 256 channels per core
G = DS // 128             # partition groups per core (2)
L = 1024                  # twist-chunk length (scan segment)
C = T // L                # chunks
F16 = mybir.dt.float16
F32 = mybir.dt.float32

_PROGRAM = None


def _build_program():
    """Build + compile the single-core Bass program (same NEFF on all cores)."""
    nc = bacc.Bacc("TRN2", target_bir_lowering=False)

    xT = nc.dram_tensor("xT", (DS, T), F16, kind="ExternalInput")
    # tabab = [cos | -sin | -sin | cos]  ([DS, 4L]); the first half doubles
    # as the twist table, the whole thing as the untwist table
    tabab = nc.dram_tensor("tabab", (DS, 4 * L), F16, kind="ExternalInput")
    rcol = nc.dram_tensor("rcol", (DS, 1), F32, kind="ExternalInput")   # r, exact f32
    bnd = nc.dram_tensor("bnd", (DS, 4), F32, kind="ExternalInput")     # Bre,Bim,-Bim,0
    eye = nc.dram_tensor("eye", (128, 128), F16, kind="ExternalInput")
    neye = nc.dram_tensor("neye", (128, 128), F16, kind="ExternalInput")
    out_re = nc.dram_tensor("out_re", (DS, T), F16, kind="ExternalOutput")
    out_im = nc.dram_tensor("out_im", (DS, T), F16, kind="ExternalOutput")

    mult = mybir.AluOpType.mult
    add = mybir.AluOpType.add
    MMF = 512  # per-matmul free dim (one PSUM bank)

    with TileContext(nc) as tc_ctx:
        with tc_ctx.tile_pool(name="tabs", bufs=1) as tpool, \
             tc_ctx.tile_pool(name="work", bufs=4) as pool, \
             tc_ctx.tile_pool(name="kpool", bufs=4) as kpool, \
             tc_ctx.tile_pool(name="psum", bufs=2, space="PSUM") as ppool:
            # twist tables + r + first x tile first, so compute starts early
            tabs = []
            x00 = None
            for g in range(G):
                pg = slice(g * 128, (g + 1) * 128)
                if g == 0:
                    x00 = pool.tile([128, L], F16, name="xt", tag="xt")
                    nc.sync.dma_start(x00[:], xT[0:128, 0:L])
                tabab_t = tpool.tile([128, 4 * L], F16, name=f"tabab{g}")
                rc_t = tpool.tile([128, 1], F32, name=f"rc{g}")
                nc.sync.dma_start(tabab_t[:, 0:2 * L], tabab[pg, 0:2 * L])
                nc.sync.dma_start(rc_t[:], rcol[pg, :])
                tabs.append({"tabab": tabab_t, "rc": rc_t})
            eye_t = tpool.tile([128, 128], F16, name="eye_t")
            neye_t = tpool.tile([128, 128], F16, name="neye_t")
            nc.sync.dma_start(eye_t[:], eye[:])
            nc.sync.dma_start(neye_t[:], neye[:])
            for g in range(G):
                pg = slice(g * 128, (g + 1) * 128)
                bnd_t = tpool.tile([128, 4], F32, name=f"bnd{g}")
                nc.sync.dma_start(tabs[g]["tabab"][:, 2 * L:4 * L],
                                  tabab[pg, 2 * L:4 * L])
                nc.sync.dma_start(bnd_t[:], bnd[pg, :])
                tabs[g]["bnd"] = bnd_t

            pieces = [(j, 0, L) for j in range(C - 1)]
            pieces += [(C - 1, 0, L // 2), (C - 1, L // 2, L)]

            K = [[None, None] for _ in range(G)]
            cur = [None] * G   # (xt, w2) per group for the current chunk
            for (j, a, b) in pieces:
                n = b - a
                for g in range(G):
                    pg = slice(g * 128, (g + 1) * 128)
                    tb = tabs[g]
                    cos_ap = tb["tabab"][:, a:b]
                    nsin_ap = tb["tabab"][:, L + a:L + b]
                    ts_ = slice(j * L + a, j * L + b)

                    if a == 0:
                        if j == 0 and g == 0:
                            xt = x00
                        else:
                            xt = pool.tile([128, L], F16, name="xt", tag="xt")
                            nc.sync.dma_start(
                                xt[:], xT[pg, j * L:(j + 1) * L])
                        w2 = pool.tile([128, 2 * L], F16, name="w2", tag="w2")
                        cur[g] = (xt, w2)
                    else:
                        xt, w2 = cur[g]

                    # twist: u = e^{-i theta l} x   (fp16 2x on DVE);
                    # one double-width op reading x twice via a repeat AP
                    u2 = pool.tile([128, 2 * n], F16, name="u2", tag="u2")
                    ure = u2[:, 0:n]
                    uim = u2[:, n:2 * n]
                    if n == L:
                        x_rep = xt[:].unsqueeze(1).broadcast_to((128, 2, L))
                        nc.vector.tensor_mul(u2[:], tb["tabab"][:, 0:2 * L],
                                             x_rep)
                    else:
                        tab_p = tb["tabab"][:, 0:2 * L].rearrange(
                            "p (s k) -> p s k", s=2)[:, :, a:b]
                        x_rep = xt[:, a:b].unsqueeze(1).broadcast_to(
                            (128, 2, n))
                        nc.vector.tensor_mul(u2[:], tab_p, x_rep)

                    # real scans, fp32 state; multiplier = broadcast r column
                    wre = w2[:, a:b]
                    wim = w2[:, L + a:L + b]
                    if a == 0:
                        init_re = 0.0 if j == 0 else K[g][0][:]
                        init_im = 0.0 if j == 0 else K[g][1][:]
                    else:
                        init_re = w2[:, a - 1:a]
                        init_im = w2[:, L + a - 1:L + a]
                    rbc = tb["rc"][:, 0:1].broadcast_to((128, n))
                    nc.vector.tensor_tensor_scan(
                        wre, rbc, ure, init_re, op0=mult, op1=add)
                    nc.vector.tensor_tensor_scan(
                        wim, rbc, uim, init_im, op0=mult, op1=add)

                    # carry rotation on ScalarE: K = e^{i theta L} W[:,L-1]
                    if b == L and j < C - 1:
                        ident = mybir.ActivationFunctionType.Identity
                        Bre = tb["bnd"][:, 0:1]
                        Bim = tb["bnd"][:, 1:2]
                        nBim = tb["bnd"][:, 2:3]
                        tmp1 = kpool.tile([128, 1], F32, name="tmp1", tag="t1k")
                        tmp2 = kpool.tile([128, 1], F32, name="tmp2", tag="t2k")
                        kre = kpool.tile([128, 1], F32, name="kre", tag="kre")
                        kim = kpool.tile([128, 1], F32, name="kim", tag="kim")
                        wreL = w2[:, L - 1:L]
                        wimL = w2[:, 2 * L - 1:2 * L]
                        nc.scalar.activation(tmp1[:], wreL, ident, scale=Bre)
                        nc.scalar.activation(kre[:], wimL, ident,
                                             scale=nBim, bias=tmp1[:])
                        nc.scalar.activation(tmp2[:], wreL, ident, scale=Bim)
                        nc.scalar.activation(kim[:], wimL, ident,
                                             scale=Bre, bias=tmp2[:])
                        K[g][0], K[g][1] = kre, kim

                    # untwist products, quad-width over repeated [wre|wim]:
                    #   t_all = [cos|nsin|nsin|cos] * [wre|wim|wre|wim]
                    #         = [t1|t2|t3|t4];  v_re = t1 + t2; v_im = -t3 + t4
                    tall = pool.tile([128, 4 * n], F16, name="tall", tag="tall")
                    t12 = tall[:, 0:2 * n]
                    t34 = tall[:, 2 * n:4 * n]
                    if n == L:
                        w_rep = w2[:].unsqueeze(1).broadcast_to((128, 2, 2 * L))
                        nc.vector.tensor_mul(tall[:], tb["tabab"][:], w_rep)
                    else:
                        w_pair = w2[:].rearrange(
                            "p (s k) -> p s k", s=2)[:, :, a:b]
                        tab12 = tb["tabab"][:, 0:2 * L].rearrange(
                            "p (s k) -> p s k", s=2)[:, :, a:b]
                        tab34 = tb["tabab"][:, 2 * L:4 * L].rearrange(
                            "p (s k) -> p s k", s=2)[:, :, a:b]
                        nc.vector.tensor_mul(t12, tab12, w_pair)
                        nc.vector.tensor_mul(t34, tab34, w_pair)

                    pre = ppool.tile([128, n], F32, name="pre", tag="pre")
                    pim = ppool.tile([128, n], F32, name="pim", tag="pim")
                    for h in range(0, n, MMF):
                        hs = slice(h, min(h + MMF, n))
                        hs2 = slice(n + h, n + min(h + MMF, n))
                        nc.tensor.matmul(pre[:, hs], eye_t[:], t12[:, hs],
                                         start=True, stop=False)
                        nc.tensor.matmul(pre[:, hs], eye_t[:], t12[:, hs2],
                                         start=False, stop=True)
                        nc.tensor.matmul(pim[:, hs], neye_t[:], t34[:, hs],
                                         start=True, stop=False)
                        nc.tensor.matmul(pim[:, hs], eye_t[:], t34[:, hs2],
                                         start=False, stop=True)
                    ore = pool.tile([128, n], F16, name="ore", tag="ore")
                    oim = pool.tile([128, n], F16, name="oim", tag="oim")
                    nc.scalar.copy(ore[:], pre[:])
                    nc.scalar.copy(oim[:], pim[:])
                    nc.sync.dma_start(out_re[pg, ts_], ore[:])
                    nc.sync.dma_start(out_im[pg, ts_], oim[:])

    nc.compile()
    return nc


def _get_program():
    global _PROGRAM
    if _PROGRAM is None:
        _PROGRAM = _build_program()
    return _PROGRAM


def _host_prep(x, size, theta):
    """Per-core input maps (host-side sharding + f64 table precompute)."""
    size64 = np.asarray(size, np.float64)
    theta64 = np.asarray(theta, np.float64)
    r = np.exp(-np.exp(size64))                        # [D]

    l64 = np.arange(L, dtype=np.float64)
    ang = theta64[:, None] * l64[None, :]              # [D, L]
    coslf = np.cos(ang).astype(np.float16)
    nsinlf = (-np.sin(ang)).astype(np.float16)
    tababf = np.concatenate([coslf, nsinlf, nsinlf, coslf], axis=1)  # [D,4L]
    rcolf = r.astype(np.float32)[:, None]
    BL = np.exp(1j * theta64 * L)                      # e^{i theta L}
    bndf = np.zeros((D, 4), np.float32)
    bndf[:, 0] = BL.real.astype(np.float32)
    bndf[:, 1] = BL.imag.astype(np.float32)
    bndf[:, 2] = (-BL.imag).astype(np.float32)

    x16T = np.ascontiguousarray(np.asarray(x, np.float32).T.astype(np.float16))
    eyef = np.eye(128, dtype=np.float16)
    in_maps = []
    for cix in range(NCORES):
        sl = slice(cix * DS, (cix + 1) * DS)
        in_maps.append({
            "xT": np.ascontiguousarray(x16T[sl]),
            "tabab": np.ascontiguousarray(tababf[sl]),
            "rcol": np.ascontiguousarray(rcolf[sl]),
            "bnd": np.ascontiguousarray(bndf[sl]),
            "eye": eyef,
            "neye": -eyef,
        })
    return in_maps


def _assemble(results):
    out = np.empty((T, D), np.complex64)
    for cix, res in enumerate(results):
        sl = slice(cix * DS, (cix + 1) * DS)
        out[:, sl] = (res["out_re"].astype(np.float32)
                      + 1j * res["out_im"].astype(np.float32)).T
    return out


def run(x, size, theta, trace=False, **spmd_kwargs):
    nc = _get_program()
    in_maps = _host_prep(x, size, theta)
    res = bass_utils.run_bass_kernel_spmd(
        nc, in_maps, core_ids=list(range(NCORES)), trace=trace, **spmd_kwargs)
    return _assemble(res.results), res


def kernel(x, size, theta):
    out, _ = run(x, size, theta, trace=False)
    return out
